# revision 56
# baseline (speedup 1.0000x reference)
"""Trainium2 Bass kernel for MinibatchDiscrimination.

Reference computation (N=256, A=1024, B=128, C=16):
    act      = (inp @ theta.reshape(A, B*C)).reshape(N, B, C)
    abs_dif  = |act[None,:,:,:] - act[:,None,:,:]|.sum(axis=3)     # [N,N,B]
    mb_feats = (exp(-abs_dif).sum(axis=0) - 1) / (N-1)             # [N,B]
    out      = concat([inp, mb_feats], axis=1)                     # [N, A+B]

Strategy (8 cores, batch-sharded on N; theta replicated):
  Every core computes the full activations act^T = (inp @ theta)^T as 16
  tiles of [128 partitions = (b,c), 256 free = j] (fp32r matmul -> bf16 +
  an exact fp32 upcast of the bf16 for per-partition scalar operands),
  then computes its own 32-row block (rows arrive pre-rolled, see below):
    - one-instruction "absdiff" units via |x| = 2*relu(x) - x, where the
      linear part sum_c x_c = S[j,b] - S[i,b] is hoisted out of the pair
      loop (S computed once by a selector matmul over act):
        DVE/POOL units: tensor_scalar(subtract, max 0) -> relu(+diff)
        ACT units:      activation(Relu, scale=-1, bias) -> relu(-diff)
    - c-reduction (sum over the 16 c's, x2): PE matmuls with 2.0-valued
      block selectors, 4-way column-tiled (tile_position) for silicon
      concurrency, accumulating d densely into PSUM [128 b', 4i x 256 j];
      one extra matmul adds the signed S[j] correction (sign sigma_b by
      engine class), and +sigma_b*S[i] rides the exp bias.  b columns are
      stored 4x4-block-permuted so each matmul sweep consumes exactly one
      freshly-DMA'd theta quarter; the store un-permutes.
    - exp + j-sum: ACT activation(Exp, scale=-1, bias, accum_out) fuses
      the exponent with the free-dim reduction.
  Core asymmetry is handled host-side: core k receives inp rolled by -32k
  rows so "my rows" are always rows 0..31 -> one static program for all
  cores, and the pairwise diagonal cancels exactly (same bf16 values),
  making the output bit-exact vs the fp32 reference for this regime
  (all off-diagonal exp(-d) underflow; the diagonal is exact).
"""

import numpy as np

N, A, B, C = 256, 1024, 128, 16
BC = B * C            # 2048
NCORES = 8
IB = N // NCORES      # 32 rows per core
NT = BC // 128        # 16 bc-tiles
KT = A // 128         # 8 contraction tiles
NJT = N // 128        # 2 row tiles of inp

# Pairwise work uses the identity |x| = 2*relu(x) - x, with
# sum_c x_c = S[j,b] - S[i,b] linear in act (computed once, not per pair):
#   d = sum_c |diff_c| = 2*sum_c relu(+-diff_c) -+ (S[j] - S[i])
# so each absdiff unit is ONE instruction:
#   DVE tiles:  tensor_scalar(subtract, max 0)   -> relu(+diff), sigma=-1
#   ACT tiles:  activation(Relu, scale=-1, bias) -> relu(-diff), sigma=+1
# The 2x rides the selector weights; sigma_b*S[j,b] is one extra matmul per
# group; +sigma_b*S[i,b] rides the exp bias.  T_ACT = which bc-tiles go to
# the scalar engine (one per column-strip so every strip keeps 3 DVE tiles).
T_ACT = frozenset({5, 10, 15})
# Of the DVE-class (relu(+diff)) units, this fraction runs on GPSIMD as a
# third engine (cost model: DVE 292ns, POOL 539ns, ACT 683ns per unit).
POOL_NUM, POOL_DEN = 1, 12
GSZ = 4               # i-rows per pairwise group
NGROUPS = IB // GSZ

_CACHE = {}


def _build():
    from contextlib import ExitStack

    import concourse.bass as bass
    import concourse.tile as tile
    from concourse import bacc, mybir

    f32 = mybir.dt.float32
    bf16 = mybir.dt.bfloat16
    i32 = mybir.dt.int32
    i16 = mybir.dt.int16
    AF = mybir.ActivationFunctionType
    OP = mybir.AluOpType

    nc = bacc.Bacc(
        "TRN2",
        target_bir_lowering=False,
        debug=False,
        enable_asserts=False,
        num_devices=NCORES,
    )

    inp_d = nc.dram_tensor("inp", [N, A], f32, kind="ExternalInput").ap()
    theta_d = nc.dram_tensor("theta", [A, BC], f32, kind="ExternalInput").ap()
    out_d = nc.dram_tensor("out", [IB, A + B], f32, kind="ExternalOutput").ap()

    with tile.TileContext(nc) as tc, ExitStack() as ctx:
        const_pool = ctx.enter_context(tc.tile_pool(name="const", bufs=1))
        data_pool = ctx.enter_context(tc.tile_pool(name="data", bufs=1))
        theta_pool = ctx.enter_context(tc.tile_pool(name="theta", bufs=16))
        ad_pool = ctx.enter_context(tc.tile_pool(name="ad", bufs=18))
        scratch_pool = ctx.enter_context(tc.tile_pool(name="scratch", bufs=4))
        ps_tr_pool = ctx.enter_context(
            tc.tile_pool(name="ps_tr", bufs=2, space=bass.MemorySpace.PSUM)
        )
        ps_act_pool = ctx.enter_context(
            tc.tile_pool(name="ps_act", bufs=2, space=bass.MemorySpace.PSUM)
        )
        ps_d_pool = ctx.enter_context(
            tc.tile_pool(name="ps_d", bufs=2, space=bass.MemorySpace.PSUM)
        )

        # ---- constants: identity (for PE transpose), block selectors ----
        iota_f128 = const_pool.tile([128, 128], f32, tag="iota_f128")
        nc.gpsimd.iota(
            iota_f128[:], pattern=[[1, 128]], channel_multiplier=0,
            allow_small_or_imprecise_dtypes=True,
        )
        iota_p = const_pool.tile([128, 1], f32, tag="iota_p")
        nc.gpsimd.iota(
            iota_p[:], pattern=[[0, 1]], channel_multiplier=1,
            allow_small_or_imprecise_dtypes=True,
        )
        ident = const_pool.tile([128, 128], f32, tag="ident")
        nc.vector.tensor_scalar(ident[:], iota_f128[:], iota_p[:], None, OP.is_equal)

        # bdiv16[p] = p // 16  (fp32)
        iota_pi = const_pool.tile([128, 1], i32, tag="iota_pi")
        nc.gpsimd.iota(iota_pi[:], pattern=[[0, 1]], channel_multiplier=1)
        bdiv16_i = const_pool.tile([128, 1], i32, tag="bdiv16_i")
        nc.vector.tensor_scalar(bdiv16_i[:], iota_pi[:], 4, None, OP.arith_shift_right)
        bdiv16 = const_pool.tile([128, 1], f32, tag="bdiv16")
        nc.vector.tensor_copy(bdiv16[:], bdiv16_i[:])

        # sel[tp][p, m] = 1.0 iff m == tp*8 + p//16   ([128, 32] bf16)
        # sel2[tp] = 2x that (for the 2*relu contributions)
        sels, sels2 = [], []
        for tp in range(4):
            colidx = const_pool.tile([128, 1], f32, tag=f"colidx{tp}")
            nc.vector.tensor_scalar_add(colidx[:], bdiv16[:], float(tp * 8))
            sel = const_pool.tile([128, 32], bf16, tag=f"sel{tp}")
            nc.vector.tensor_scalar(
                sel[:], iota_f128[:, 0:32], colidx[:], None, OP.is_equal
            )
            sels.append(sel)
            sel2 = const_pool.tile([128, 32], bf16, tag=f"sel2_{tp}")
            nc.vector.tensor_scalar_mul(sel2[:], sel[:], 2.0)
            sels2.append(sel2)

        # sigma_b: +1 where b's tile is ACT-assigned, -1 otherwise.  b's tile
        # is t = b//8; t in {5,10,15} <=> ((t ^ (t>>2)) & 3) == 0 and t != 0.
        bdiv8_i = const_pool.tile([128, 1], i32, tag="bdiv8_i")
        nc.vector.tensor_scalar(bdiv8_i[:], iota_pi[:], 3, None, OP.arith_shift_right)
        tsr2_i = const_pool.tile([128, 1], i32, tag="tsr2_i")
        nc.vector.tensor_scalar(tsr2_i[:], bdiv8_i[:], 2, None, OP.arith_shift_right)
        tx_i = const_pool.tile([128, 1], i32, tag="tx_i")
        nc.vector.tensor_tensor(tx_i[:], bdiv8_i[:], tsr2_i[:], OP.bitwise_xor)
        tu_i = const_pool.tile([128, 1], i32, tag="tu_i")
        nc.vector.tensor_scalar(tu_i[:], tx_i[:], 3, None, OP.bitwise_and)
        tu_f = const_pool.tile([128, 1], f32, tag="tu_f")
        nc.vector.tensor_copy(tu_f[:], tu_i[:])
        bdiv8_f = const_pool.tile([128, 1], f32, tag="bdiv8_f")
        nc.vector.tensor_copy(bdiv8_f[:], bdiv8_i[:])
        e0 = const_pool.tile([128, 1], f32, tag="e0")
        nc.vector.tensor_scalar(e0[:], tu_f[:], 0.0, None, OP.is_equal)
        ez = const_pool.tile([128, 1], f32, tag="ez")
        nc.vector.tensor_scalar(ez[:], bdiv8_f[:], 0.0, None, OP.is_equal)
        sig2 = const_pool.tile([128, 1], f32, tag="sig2")
        nc.vector.tensor_tensor(sig2[:], e0[:], ez[:], OP.subtract)
        sig2b = const_pool.tile([128, 1], f32, tag="sig2b")
        nc.vector.tensor_scalar(sig2b[:], sig2[:], 2.0, None, OP.mult)
        sig_col = const_pool.tile([128, 1], f32, tag="sig_col")
        nc.vector.tensor_scalar(sig_col[:], sig2b[:], 1.0, None, OP.subtract)
        ident_sig = const_pool.tile([128, 128], bf16, tag="ident_sig")
        nc.vector.tensor_scalar(
            ident_sig[:], ident[:], sig_col[:], None, OP.mult
        )
        ident_neg = const_pool.tile([128, 128], bf16, tag="ident_neg")
        nc.vector.tensor_scalar(ident_neg[:], ident[:], -1.0, None, OP.mult)

        # ---- load inp, build inpT via PE transpose ----
        inp_sb = data_pool.tile([128, NJT, A], f32, tag="inp_sb")
        for jt in range(NJT):
            for kc in range(2):
                nc.sync.dma_start(
                    inp_sb[:, jt, kc * (A // 2):(kc + 1) * (A // 2)],
                    inp_d[jt * 128:(jt + 1) * 128,
                          kc * (A // 2):(kc + 1) * (A // 2)],
                )
        f32r = mybir.dt.float32r
        inpT = data_pool.tile([128, KT, N], f32r, tag="inpT")
        for kt in range(KT):
            for jt in range(NJT):
                ps_t = ps_tr_pool.tile([128, 128], f32, tag="ps_t")
                nc.tensor.transpose(
                    ps_t[:], inp_sb[:, jt, kt * 128:(kt + 1) * 128], ident[:]
                )
                nc.scalar.copy(inpT[:, kt, jt * 128:(jt + 1) * 128], ps_t[:])

        # ---- act matmul, streamed per 4-tile column slab ----
        # theta is loaded in [128, 512] slabs (2KB contiguous rows) to keep
        # the DMA descriptor count low; each slab covers 4 bc-tiles.
        act_bf = data_pool.tile([128, NT, N], bf16, tag="act_bf")
        act_f32 = data_pool.tile([128, NT, N], f32, tag="act_f32")
        TQ = 4                      # bc-tiles per theta slab
        for q in range(NT // TQ):
            slabs = []
            for kt in range(KT):
                th = theta_pool.tile([128, TQ * 128], f32r, tag="th")
                nc.sync.dma_start(
                    th[:],
                    theta_d[kt * 128:(kt + 1) * 128,
                            q * TQ * 128:(q + 1) * TQ * 128].bitcast(f32r),
                )
                slabs.append(th)
            for tq in range(TQ):
                t = q * TQ + tq
                ps_a = ps_act_pool.tile([128, N], f32, tag="ps_a")
                for kt in range(KT):
                    nc.tensor.matmul(
                        ps_a[:],
                        slabs[kt][:, tq * 128:(tq + 1) * 128],
                        inpT[:, kt, :],
                        start=(kt == 0), stop=(kt == KT - 1),
                    )
                nc.scalar.copy(act_bf[:, t, :], ps_a[:])
                nc.vector.tensor_copy(act_f32[:, t, :], act_bf[:, t, :])

        # ---- S[b, j] = sum_c act[j, b, c]  (one column-tiled pass) ----
        ps_s = ps_act_pool.tile([128, N], f32, tag="ps_a")
        for tpn in range(4):
            for g in range(4):
                t = g + 4 * tpn
                nc.tensor.matmul(
                    ps_s[32 * g:32 * g + 32, :], sels[tpn][:], act_bf[:, t, :],
                    start=(tpn == 0), stop=(tpn == 3),
                    tile_position=(0, 32 * g), skip_group_check=True,
                )
        S_sb = data_pool.tile([128, N], bf16, tag="S_sb")
        nc.vector.tensor_copy(S_sb[:], ps_s[:])
        # Ssig[b, j] = sigma_b * S[b, j], fp32 (exp bias source; exact upcast
        # of the bf16 values so the diagonal cancels exactly)
        Ssig = data_pool.tile([128, N], f32, tag="Ssig")
        nc.vector.tensor_scalar(Ssig[:], S_sb[:], sig_col[:], None, OP.mult)
        S_neg = data_pool.tile([128, N], f32, tag="S_neg")
        nc.vector.tensor_scalar(S_neg[:], S_sb[:], -1.0, None, OP.mult)

        # ---- pairwise: relu units -> c-reduce + S-corr (PE) -> exp ----
        mb = data_pool.tile([128, IB], f32, tag="mb")
        # b-columns of d are stored permuted: tile t lands in column strip
        # g = t % 4, slot t // 4 (the 4x4 transpose pi; T_ACT are fixed
        # points of pi so sigma is unchanged).  Each tp-sweep then consumes
        # one theta quarter, pipelining with the DMA stream.
        TSEQ = list(range(NT))
        unit_no = 0
        NOACT_FROM = NGROUPS - 2
        for gi in range(NGROUPS):
            gi_noact = gi >= NOACT_FROM
            ps_d = ps_d_pool.tile([128, GSZ * N], f32, tag="ps_d")
            ad_tiles = {}
            for t in TSEQ:
                ad = ad_pool.tile([128, GSZ * N], bf16, tag="ad")
                for il in range(GSZ):
                    i = gi * GSZ + il
                    dst = ad[:, il * N:(il + 1) * N]
                    if t in T_ACT and not gi_noact:
                        nc.scalar.activation(
                            dst, act_bf[:, t, :], AF.Relu,
                            bias=act_f32[:, t, i:i + 1], scale=-1.0,
                        )
                    else:
                        eng = (nc.gpsimd
                               if (unit_no % POOL_DEN) < POOL_NUM
                               else nc.vector)
                        eng.tensor_scalar(
                            dst, act_bf[:, t, :], act_f32[:, t, i:i + 1],
                            0.0, OP.subtract, OP.max,
                        )
                        unit_no += 1
                ad_tiles[t] = ad
            # c-reduce: 4-way column-tiled selector matmuls (weight 2.0),
            # one PSUM bank (N=512) per matmul; then the signed S correction
            for tpn in range(4):
                for g in range(4):
                    t = g + 4 * tpn
                    for half in range(GSZ * N // 512):
                        nc.tensor.matmul(
                            ps_d[32 * g:32 * g + 32,
                                 half * 512:(half + 1) * 512],
                            sels2[tpn][:],
                            ad_tiles[t][:, half * 512:(half + 1) * 512],
                            start=(tpn == 0), stop=False,
                            tile_position=(0, 32 * g),
                            skip_group_check=True,
                        )
            S_rep = S_sb[:].rearrange("p (o j) -> p o j", o=1).broadcast_to(
                [128, 2, N]
            )
            corr = ident_neg if gi_noact else ident_sig
            for half in range(GSZ * N // 512):
                nc.tensor.matmul(
                    ps_d[:, half * 512:(half + 1) * 512],
                    corr[:], S_rep,
                    start=False, stop=True, skip_group_check=True,
                )
            bias_src = S_neg if gi_noact else Ssig
            for il in range(GSZ):
                i = gi * GSZ + il
                scr = scratch_pool.tile([128, N], bf16, tag="scr")
                nc.scalar.activation(
                    scr[:], ps_d[:, il * N:(il + 1) * N], AF.Exp,
                    scale=-1.0, bias=bias_src[:, i:i + 1],
                    accum_out=mb[:, i:i + 1],
                )

        # ---- finalize: (sum - 1) / 255, transpose to [32 i, 128 b],
        # un-permute the b columns, store.  Done in two i-halves so the tail
        # after the last exp is short. ----
        mb2 = data_pool.tile([128, IB], f32, tag="mb2")
        H = IB // 2
        for h in range(2):
            sl = slice(h * H, (h + 1) * H)
            nc.vector.tensor_scalar(
                mb2[:, sl], mb[:, sl], 1.0, 1.0 / (N - 1), OP.subtract, OP.mult
            )
            ps_mbT = ps_tr_pool.tile([H, 128], f32, tag="ps_t")
            nc.tensor.transpose(ps_mbT[:], mb2[:, sl], ident[:])
            mbT_h = data_pool.tile([H, B], f32, tag=f"mbT{h}")
            nc.scalar.copy(mbT_h[:], ps_mbT[:])
            mbT_fx = data_pool.tile([H, B], f32, tag=f"mbTf{h}")
            mb_src = mbT_h[:].rearrange(
                "p (v u e) -> p v u e", v=4, u=4, e=8
            ).transpose([0, 2, 1, 3])
            mb_dst = mbT_fx[:].rearrange(
                "p (u v e) -> p u v e", u=4, v=4, e=8
            )
            nc.vector.tensor_copy(mb_dst, mb_src)
            nc.sync.dma_start(out_d[sl, A:A + B], mbT_fx[:])
        # passthrough of this core's own inp rows
        nc.sync.dma_start(out_d[:, 0:A], inp_d[0:IB, :])

    nc.compile()
    return nc


def _get_nc():
    if "nc" not in _CACHE:
        _CACHE["nc"] = _build()
    return _CACHE["nc"]


def kernel(inp: np.ndarray, theta: np.ndarray) -> np.ndarray:
    from concourse.bass_utils import run_bass_kernel_spmd

    nc = _get_nc()
    inp = np.ascontiguousarray(np.asarray(inp, dtype=np.float32))
    theta_r = np.ascontiguousarray(
        np.asarray(theta, dtype=np.float32).reshape(A, BC)
    )
    in_maps = [
        {"inp": np.ascontiguousarray(np.roll(inp, -IB * k, axis=0)),
         "theta": theta_r}
        for k in range(NCORES)
    ]
    res = run_bass_kernel_spmd(nc, in_maps, core_ids=list(range(NCORES)))
    return np.concatenate([r["out"] for r in res.results], axis=0)
